# revision 36
# baseline (speedup 1.0000x reference)
"""BoltzmannRouter Trainium2 kernel: 8-core data-parallel Bass implementation.

Full inputs: x (4, 4096, 2048) f32, gate_w (64, 2048) f32.
Output: routing weights (4, 4096, 64) f32 (softmax -> top-44 mask -> renorm).

Sharding: 16384 tokens split 2048/core across 8 NeuronCores; gate weight
replicated.

Per-core pipeline (8 slabs x 256 tokens):
  - x shipped as fp16 (single precision-split only on the gate weight, which
    is packed [wh|wl] so one matmul pass yields both the fp16-high scores and
    the low-order correction): halves HBM traffic AND PE passes vs fp16x3.
  - One DMA per slab ([128, 16kc, 256t] fp16, 8 KiB/partition) -> 17 total
    dma_starts on SP instead of 69 (SP issue is ~600ns each).
  - PE: 16 accumulating matmuls -> scores*64 in PSUM [2E, 256]; DVE combines
    hi+lo (one STT); PE transposes to token-major via -1/64-scaled identity.
  - Softmax WITHOUT max-shift (|scores| <~ 4, exp is safe in fp32); the
    top-44 mask is applied in the u=exp(scores) domain: threshold
    u >= exp(s_(21)) computed by ScalarE from the DVE max8-round output, so
    the mask+renormalize (mask, sum, divide) runs on the idle GpSimd engine
    (scalar_tensor_tensor with accumulate + normalize_recip).
  - EPS term dropped: reference adds 1e-8 to a ~0.8 denominator (rel 1e-8,
    sub-ulp vs the fp16 input quantization at ~5e-3 rel).
"""

import os
import sys

sys.path.insert(0, "/opt/trn_rl_repo")

import numpy as np

D = 2048
E = 64
EPS = 1e-8
NEG_BIG = -1e30
TEMPERATURE = 2.718281828459045
N_CORES = 8
TPC = 2048  # tokens per core
SLAB = 256  # tokens per slab (one x DMA, one PSUM scores tile)
N_SLABS = TPC // SLAB
KC = D // 128

W_SCALE = 64.0  # 2^6: lifts gate_w into fp16-normal range
LO_SCALE = 4096.0  # 2^12: scale on the low fp16 split part of gate_w

# 256-token slabs, graduated at both ends: the first slabs are small so the
# pipeline head starts as soon as possible (concurrent DMAs on one queue
# interleave descriptors, so slab 0's completion otherwise waits on the
# prefetched slabs too), and the last is split so the serial selection tail
# after the final x DMA halves.  Host packing and the device program must
# agree on this list.
WIDTHS = [128, 128] + [SLAB] * (N_SLABS - 2) + [SLAB // 2, SLAB // 2]


def _build_nc():
    import concourse.bacc as bacc
    import concourse.mybir as mybir
    from concourse.tile import TileContext

    F32 = mybir.dt.float32
    F16 = mybir.dt.float16

    lean_tail = os.environ.get("BOLTZ_LEAN_TAIL", "1") == "1"
    if lean_tail:
        # the stock Tile exit emits drain + barrier + sem-clear + barrier
        # (~8us); the kernel preamble already range-clears the semaphores at
        # the start of every execution, so drain + one barrier suffices
        barrier = os.environ.get("BOLTZ_TAIL_BARRIER", "0") == "1"

        def _lean_drain_and_barrier(self, tick_clock, wait_clock):
            from concourse.tile import ScopedClock

            drain_inst = self.nc.sync.drain()
            wait_clock.add_sem_waits(
                drain_inst.ins, ScopedClock({None: tick_clock.global_clock})
            )
            if barrier:
                self.nc.all_engine_barrier()
            popped = self.nc._tile_sem_poison_stack.pop()
            assert popped is self._sem_poison
            self.sems.allocated()

        TileContext._drain_and_barrier = _lean_drain_and_barrier

    # the preamble's per-engine SEMAPHORE_RANGE_CLEAR costs time proportional
    # to the kernel sem range (106 sems ~ 3us); this kernel allocates ~17,
    # so shrink the range (with 2x margin) to cut the fixed startup cost
    import concourse.bass as cbass
    import concourse.env as cenv

    base = cenv.get_walrus_max_sem_num()
    cbass.get_kernel_semaphore_range = lambda: range(base, base + 40)

    nc = bacc.Bacc(None, target_bir_lowering=False)
    # host-packed layouts (see kernel() below):
    #   xpk[p, s*4096 + kc*256 + t] = fp16(x_shard[token s*256+t, d kc*128+p])
    #   whl[p, kc*128 + e]      = wh[kc*128+p, e]        e in [0, 64)
    #   whl[p, kc*128 + 64 + e] = wl[kc*128+p, e]
    xpk_d = nc.declare_dram_parameter("xpk", [128, N_SLABS * KC * SLAB], F16,
                                      isOutput=False)
    whl_d = nc.declare_dram_parameter("whl", [128, KC * 2 * E], F16,
                                      isOutput=False)
    # out[p, j*64 + e] = weight(token j*128+p, e), j in [0, 16)
    out_d = nc.declare_dram_parameter("out", [128, (TPC // 128) * E], F32,
                                      isOutput=True)

    mult = mybir.AluOpType.mult
    is_le = mybir.AluOpType.is_le
    Exp = mybir.ActivationFunctionType.Exp

    with TileContext(nc) as tc:
        with (
            tc.tile_pool(name="const", bufs=1) as cpool,
            tc.tile_pool(name="xg", bufs=4) as xpool,
            tc.tile_pool(name="sneg", bufs=3) as spool,
            tc.tile_pool(name="uy", bufs=3) as uypool,
            tc.tile_pool(name="og", bufs=3) as ogpool,
            tc.tile_pool(name="small", bufs=8) as smpool,
            tc.tile_pool(name="ps_s", bufs=3, space="PSUM") as ps_s_pool,
            tc.tile_pool(name="ps_t", bufs=4, space="PSUM") as ps_t_pool,
        ):
            # combiner [2E, E]: rows 0:64 diag(-1/W_SCALE), rows 64:128
            # diag(-1/(W_SCALE*LO_SCALE)) — the token-major transpose matmul
            # then also merges the hi/lo score parts, descales and negates
            comb = cpool.tile([2 * E, E], F32)
            nc.gpsimd.memset(comb, 0.0)
            nc.gpsimd.affine_select(
                out=comb,
                in_=comb,
                compare_op=mybir.AluOpType.not_equal,
                fill=-1.0 / W_SCALE,
                base=0,
                pattern=[[-1, E]],
                channel_multiplier=1,
            )
            nc.gpsimd.affine_select(
                out=comb,
                in_=comb,
                compare_op=mybir.AluOpType.not_equal,
                fill=-1.0 / (W_SCALE * LO_SCALE),
                base=-E,
                pattern=[[-1, E]],
                channel_multiplier=1,
            )

            whl_sb = cpool.tile([128, KC, 2 * E], F16)
            nc.sync.dma_start(out=whl_sb, in_=whl_d[:, :])

            def do_tail(off, w, sc):
                """Transpose + softmax + top-44 mask + renorm + out DMA for
                one slab whose combined scores sit in SBUF `sc` [2E, w]."""
                nj = w // 128
                # token-major negated scores [128 tok, nj, 64 e]; the comb
                # stationary merges hi+lo/LO_SCALE and scales by -1/W_SCALE
                # (full-width PSUM tile, sliced for narrow slabs: PSUM tiles
                # are bank-granular so per-width tags would double the banks)
                pst_full = ps_t_pool.tile([128, 2, E], F32, tag="pst")
                pst = pst_full[:, :nj, :]
                for j in range(nj):
                    nc.tensor.matmul(
                        pst[:, j, :],
                        lhsT=sc[:, j * 128 : (j + 1) * 128],
                        rhs=comb,
                    )

                # Scalar stages everything DVE needs into SBUF so pst (PSUM)
                # frees right after; y2 is a pristine copy for the mask
                # compare, y is the working copy the match_replaces destroy.
                y = uypool.tile([128, nj, E], F32, tag=f"y{nj}")
                y2 = uypool.tile([128, nj, E], F32, tag=f"y2{nj}")
                u = uypool.tile([128, nj, E], F32, tag=f"u{nj}")
                wm = uypool.tile([128, nj, E], F32, tag=f"wm{nj}")
                ws = smpool.tile([128, nj], F32, tag=f"ws{nj}")
                og = ogpool.tile([128, nj, E], F32, tag=f"og{nj}")

                nc.scalar.copy(y, pst)
                nc.scalar.copy(y2, pst)
                for j in range(nj):
                    # u = exp(scores); no max-shift needed (|scores| small)
                    nc.scalar.activation(u[:, j, :], pst[:, j, :], Exp,
                                         scale=-1.0)
                for j in range(nj):
                    yj = y[:, j, :]
                    # bottom-20 threshold: top-8 of -scores twice removed,
                    # then rank 17-24; index 4 = 21st smallest score
                    r1 = smpool.tile([128, 8], F32, tag="r1")
                    nc.vector.max(r1, yj)
                    nc.vector.match_replace(yj, r1, yj, NEG_BIG)
                    r2 = smpool.tile([128, 8], F32, tag="r2")
                    nc.vector.max(r2, yj)
                    nc.vector.match_replace(yj, r2, yj, NEG_BIG)
                    r3 = smpool.tile([128, 8], F32, tag="r3")
                    nc.vector.max(r3, yj)
                    # wm = u * (-scores <= thr); ws = sum(wm).  Score-domain
                    # compare (not u-domain) so the Exp table's quantization
                    # cannot flip near-boundary mask decisions.
                    nc.vector.scalar_tensor_tensor(
                        out=wm[:, j, :],
                        in0=y2[:, j, :],
                        scalar=r3[:, 4:5],
                        in1=u[:, j, :],
                        op0=is_le,
                        op1=mult,
                        accum_out=ws[:, j : j + 1],
                    )
                    nc.gpsimd.normalize_recip(
                        og[:, j, :], wm[:, j, :], ws[:, j : j + 1]
                    )

                col = (off // 128) * E
                nc.sync.dma_start(
                    out=out_d[:, col : col + nj * E], in_=og
                )

            # all x dma_starts are emitted BEFORE any out DMA: SP executes
            # its queue in order, and an out DMA's semaphore wait (og ready
            # only after the whole selection chain) would otherwise block
            # the issue of later x slabs and starve the entire pipeline
            xs_tiles = []
            off = 0
            for si, w in enumerate(WIDTHS):
                xs = xpool.tile([128, KC, w], F16, tag=f"xs{w}")
                nc.sync.dma_start(
                    out=xs, in_=xpk_d[:, off * KC : off * KC + KC * w]
                )
                xs_tiles.append(xs)
                off += w

            pending = None
            off = 0
            for si, w in enumerate(WIDTHS):
                xs = xs_tiles[si]

                # rows 0:64 = wh.T@x (64*scores, fp16-high), 64:128 = wl.T@x
                ps1_full = ps_s_pool.tile([2 * E, SLAB], F32, tag="ps1")
                ps1 = ps1_full[:, :w]
                for kc in range(KC):
                    nc.tensor.matmul(
                        ps1, lhsT=whl_sb[:, kc, :], rhs=xs[:, kc, :],
                        start=(kc == 0), stop=(kc == KC - 1),
                    )

                # software pipeline: the previous slab's transpose/selection
                # is emitted here so the PE rolls straight from this slab's
                # score matmuls into the previous slab's transposes with its
                # dependencies long satisfied (keeps the PE DVFS ramp alive)
                if pending is not None:
                    do_tail(*pending)

                # PSUM can only feed one operand per DVE op and the PE needs
                # SBUF stationaries, so stage the raw hi/lo scores in SBUF
                sc = spool.tile([2 * E, w], F32, tag=f"sc{w}")
                nc.scalar.copy(sc, ps1)
                if si <= 1:
                    # head: run the tail immediately — the PE is DMA-bound
                    # here anyway, and the one-slab delay would push the
                    # entire DVE chain (the steady-state pacer) ~2us later
                    do_tail(off, w, sc)
                else:
                    pending = (off, w, sc)
                off += w

            do_tail(*pending)

    nc.finalize()
    return nc


_NC = None
LAST_EXEC_NS = None
LAST_RESULTS = None


def _get_nc():
    global _NC
    if _NC is None:
        _NC = _build_nc()
    return _NC


def _pack_inputs(x, gate_w):
    x = np.asarray(x)
    gate_w = np.asarray(gate_w)
    Btot = x.shape[0] * x.shape[1]
    x2 = np.ascontiguousarray(x.reshape(Btot, D).astype(np.float32, copy=False))

    wt = (gate_w.astype(np.float32, copy=False).T
          * np.float32(W_SCALE / TEMPERATURE))  # [D, E], scaled by 64/T
    wh = wt.astype(np.float16)
    wl = ((wt - wh.astype(np.float32)) * np.float32(LO_SCALE)).astype(np.float16)
    whl = np.concatenate([wh, wl], axis=1)  # [D, 2E]
    whl = np.ascontiguousarray(
        whl.reshape(KC, 128, 2 * E).transpose(1, 0, 2).reshape(128, KC * 2 * E)
    )

    in_maps = []
    for i in range(Btot // TPC):
        shard16 = x2[i * TPC : (i + 1) * TPC].astype(np.float16)  # [TPC, D]
        cols = []
        off = 0
        for w in WIDTHS:
            blk = shard16[off : off + w]  # [w, D]
            cols.append(
                blk.T.reshape(KC, 128, w).transpose(1, 0, 2).reshape(128, KC * w)
            )
            off += w
        xpk = np.ascontiguousarray(np.concatenate(cols, axis=1))
        in_maps.append({"xpk": xpk, "whl": whl})
    return in_maps


def _unpack_out(res, batch_shape):
    outs = []
    for i in range(N_CORES):
        o = res.results[i]["out"].reshape(128, TPC // 128, E)
        outs.append(o.transpose(1, 0, 2).reshape(TPC, E))
    return np.concatenate(outs, axis=0).reshape(*batch_shape, E)


def kernel(x, gate_w, trace=False):
    global LAST_EXEC_NS, LAST_RESULTS
    from concourse.bass_utils import run_bass_kernel_spmd

    x = np.asarray(x)
    in_maps = _pack_inputs(x, gate_w)
    nc = _get_nc()

    kwargs = {}
    if trace:
        try:
            import antenv.axon_hooks  # noqa: F401  (shimmed by tracehook)

            kwargs["trace"] = True
        except ImportError:
            pass
    res = run_bass_kernel_spmd(nc, in_maps, core_ids=list(range(N_CORES)), **kwargs)
    LAST_EXEC_NS = res.exec_time_ns
    LAST_RESULTS = res
    return _unpack_out(res, x.shape[:2])
